# revision 25
# baseline (speedup 1.0000x reference)
"""Multi-head causal self-attention on 8 Trainium2 NeuronCores.

Problem: B=256, T=256, E=384, H=6, D=64 (fp32 in/out).
Strategy: pure data parallelism over the batch dim - each of the 8 cores
processes 32 batches end-to-end. No collectives.

v3 design:
- all matmul operands bf16 (1 cyc/row on PE at any free-dim size, fp32
  PSUM accumulation); inputs pre-cast host-side.
- x^T computed on the HOST (free) and DMA'd in directly - no on-device
  x transpose at all.
- o^T via PE transpose (bf16, 128 cyc) + DVE copy; DMA xbar transposes
  proved to serialize the whole DMA path (xbar-mode thrash), so none.
- scores: per-head PSUM bank [s0 256 | s1 128] -> single fused exp per
  head on ACT (scale=1/8, bf16 out). The two MMs of one head share a PE
  row-group so they serialize; concurrent different-row-group MMs
  (adjacent heads, partition offsets 0/64) land in different banks.
  (Concurrent MMs into one PSUM bank crash the HW - found the hard way.)
- causal mask on diag blocks via 2 broadcast tensor_tensor muls (DVE).
- softmax denominator via packed [v_h | 1] rhs (65th column) in the av
  matmul; normalization fused into the PSUM->SBUF copy as a broadcast
  tensor_tensor mul with the per-head reciprocal.
- q/k/v/proj PSUM banks packed 2 chunks per bank (K=128 MMs serialize).

Softmax max-subtraction skipped deliberately: |scores| < ~2 given the
input distribution, exp() is well-conditioned in fp32->bf16.
"""

import os
import sys

import numpy as np

sys.path.insert(0, "/opt/trn_rl_repo")

B, T, E, H, D = 256, 256, 384, 6, 64
HD = H * D  # 384
N_CORES = 8
BL = B // N_CORES  # 32 batches per core

NORM = os.environ.get("K_NORM", "bcast")  # bcast | ts


def _build_program(n_batches=BL, reps=1):
    import concourse.mybir as mybir
    import concourse.tile as tile
    from concourse import bacc

    FP = mybir.dt.float32
    BF = mybir.dt.bfloat16
    AF = mybir.ActivationFunctionType

    nc = bacc.Bacc(
        "TRN2",
        target_bir_lowering=False,
        debug=False,
        enable_asserts=False,
        num_devices=N_CORES,
        enable_partition_id=False,
    )

    # x arrives pre-transposed per batch: [nb*E, T]
    x_d = nc.dram_tensor("x", (n_batches * E, T), BF, kind="ExternalInput").ap()
    wq_d = nc.dram_tensor("wq", (E, HD), BF, kind="ExternalInput").ap()
    wk_d = nc.dram_tensor("wk", (E, HD), BF, kind="ExternalInput").ap()
    wv_d = nc.dram_tensor("wv", (E, HD), BF, kind="ExternalInput").ap()
    wo_d = nc.dram_tensor("wo", (HD, E), BF, kind="ExternalInput").ap()
    bo_d = nc.dram_tensor("bo", (128, E), FP, kind="ExternalInput").ap()
    mk_d = nc.dram_tensor("mask", (128, 128), BF, kind="ExternalInput").ap()
    id_d = nc.dram_tensor("ident", (128, 128), BF, kind="ExternalInput").ap()
    y_d = nc.dram_tensor("y", (n_batches * T, E), FP, kind="ExternalOutput").ap()

    with tile.TileContext(nc) as tc:
        from contextlib import ExitStack

        with ExitStack() as ctx:
            const = ctx.enter_context(tc.tile_pool(name="const", bufs=1))
            wq_t = const.tile([128, 3, HD], BF, tag="wq")
            wk_t = const.tile([128, 3, HD], BF, tag="wk")
            wv_t = const.tile([128, 3, HD], BF, tag="wv")
            wo_t = const.tile([128, 3, E], BF, tag="wo")
            bo_t = const.tile([128, E], FP, tag="bo")
            mk_t = const.tile([128, 128], BF, tag="mask")
            id_t = const.tile([128, 128], BF, tag="ident")
            # SBUF pools
            xTp = ctx.enter_context(tc.tile_pool(name="xT", bufs=3))

            # wq + batch-0 xT first so the first q matmuls start ASAP;
            # remaining constants follow (needed later in the pipeline).
            nc.sync.dma_start(wq_t[:], wq_d.rearrange("(c p) n -> p c n", p=128))
            xT0 = xTp.tile([128, 3, T], BF, tag="xT", name="xT0")
            nc.sync.dma_start(
                xT0[:], x_d[0:E, :].rearrange("(c p) n -> p c n", p=128)
            )
            for t_, d_ in ((wk_t, wk_d), (wv_t, wv_d), (wo_t, wo_d)):
                nc.sync.dma_start(t_[:], d_.rearrange("(c p) n -> p c n", p=128))
            nc.sync.dma_start(bo_t[:], bo_d)
            nc.sync.dma_start(mk_t[:], mk_d)
            nc.sync.dma_start(id_t[:], id_d)
            mk_b = mk_t[:].unsqueeze(1).broadcast_to((128, H, 128))
            qkp = ctx.enter_context(tc.tile_pool(name="qk", bufs=2))
            vpp = ctx.enter_context(tc.tile_pool(name="vp", bufs=2))
            exp = ctx.enter_context(tc.tile_pool(name="ex", bufs=2))
            rcp = ctx.enter_context(tc.tile_pool(name="rc", bufs=2))
            ocp = ctx.enter_context(tc.tile_pool(name="oc", bufs=2))
            oTp = ctx.enter_context(tc.tile_pool(name="oT", bufs=2))
            fip = ctx.enter_context(tc.tile_pool(name="fi", bufs=3))

            # PSUM pools: 4 + 2 + 2 = 8 banks
            ps = ctx.enter_context(tc.tile_pool(name="ps", bufs=4, space="PSUM"))
            ps2 = ctx.enter_context(tc.tile_pool(name="ps2", bufs=2, space="PSUM"))
            pst = ctx.enter_context(tc.tile_pool(name="pst", bufs=2, space="PSUM"))

            def _one_batch(b):
                # ---- xT [e, t] bf16: plain DMA load (pre-transposed on host) ----
                if b == 0:
                    xTt = xT0
                else:
                    xTt = xTp.tile([128, 3, T], BF, tag="xT")
                    nc.sync.dma_start(
                        xTt[:],
                        x_d[b * E : (b + 1) * E, :].rearrange(
                            "(c p) n -> p c n", p=128
                        ),
                    )

                # ---- q/k projections (PSUM banks packed 2 chunks each) ----
                qT = qkp.tile([128, 3, T], BF, tag="qT")
                kT = qkp.tile([128, 3, T], BF, tag="kT")
                pA = ps.tile([128, 512], FP, tag="ps")  # q hc0 | q hc1
                pB = ps.tile([128, 512], FP, tag="ps")  # q hc2 | k hc0
                pC = ps.tile([128, 512], FP, tag="ps")  # k hc1 | k hc2
                mm_plan = [
                    (pA, 0, wq_t, 0),
                    (pA, 1, wq_t, 1),
                    (pB, 0, wq_t, 2),
                    (pB, 1, wk_t, 0),
                    (pC, 0, wk_t, 1),
                    (pC, 1, wk_t, 2),
                ]
                for pt_, half, w_t, hc in mm_plan:
                    for ec in range(3):
                        nc.tensor.matmul(
                            pt_[:, half * T : half * T + T],
                            w_t[:, ec, hc * 128 : hc * 128 + 128],
                            xTt[:, ec, :],
                            start=(ec == 0),
                            stop=(ec == 2),
                        )
                # copy-cast PSUM->SBUF bf16 on ACT
                nc.scalar.copy(qT[:, 0:2, :], pA[:])
                nc.scalar.copy(qT[:, 2, :], pB[:, 0:T])
                nc.scalar.copy(kT[:, 0, :], pB[:, T : 2 * T])
                nc.scalar.copy(kT[:, 1:3, :], pC[:])

                # ---- v projection -> packed [v_h | 1] per head ----
                vp = vpp.tile([128, 2, H * 65], BF, tag="vp")
                for sc in range(2):
                    pv = ps2.tile([128, 390], FP, tag="ps2")
                    for ec in range(3):
                        nc.tensor.matmul(
                            pv[:, 0:HD],
                            xTt[:, ec, sc * 128 : sc * 128 + 128],
                            wv_t[:, ec, :],
                            start=(ec == 0),
                            stop=(ec == 2),
                        )
                    dst = vp[:, sc, :].rearrange("p (h c) -> p h c", c=65)
                    nc.vector.tensor_copy(
                        dst[:, :, 0:64], pv[:, 0:HD].rearrange("p (h d) -> p h d", d=64)
                    )
                    nc.vector.memset(dst[:, :, 64:65], 1.0)


                # ---- scores + exp: per-head PSUM bank [s0 256 | s1 128] ----
                # Both MMs of a head share a row-group (same po) so they
                # serialize on the PE; concurrent different-row-group MMs
                # (adjacent heads) land in different banks. Concurrent MMs
                # into one bank crash the HW.
                ex = exp.tile([128, H, 384], BF, tag="ex")
                for h in range(H):
                    hc, po = divmod(h, 2)
                    po *= 64
                    sb = ps.tile([128, 512], FP, tag="ps", name=f"sb{h}")
                    nc.tensor.matmul(
                        sb[:, 0:T],
                        kT[po : po + 64, hc, 0:128],
                        qT[po : po + 64, hc, :],
                        start=True,
                        stop=True,
                    )
                    nc.tensor.matmul(
                        sb[:, T : T + 128],
                        kT[po : po + 64, hc, 128:256],
                        qT[po : po + 64, hc, 128:256],
                        start=True,
                        stop=True,
                    )
                    nc.scalar.activation(
                        ex[:, h, :], sb[:, 0 : T + 128], AF.Exp, scale=0.125
                    )
                # causal mask on diagonal blocks (s0 diag at cols 0:128,
                # s1 diag at cols 256:384)
                nc.vector.tensor_mul(ex[:, :, 0:128], ex[:, :, 0:128], mk_b)
                nc.vector.tensor_mul(ex[:, :, 256:384], ex[:, :, 256:384], mk_b)


                # ---- av + normalize -> oc[t, hd] ----
                oc = ocp.tile([128, 2, HD], BF, tag="oc")
                for tc_ in range(2):
                    pav = ps2.tile([128, 390], FP, tag="ps2")
                    for h in range(H):
                        sl = pav[:, h * 65 : h * 65 + 65]
                        if tc_ == 0:
                            nc.tensor.matmul(
                                sl,
                                ex[:, h, 0:128],
                                vp[:, 0, h * 65 : h * 65 + 65],
                                start=True,
                                stop=True,
                            )
                        else:
                            nc.tensor.matmul(
                                sl,
                                ex[:, h, 128:256],
                                vp[:, 0, h * 65 : h * 65 + 65],
                                start=True,
                                stop=False,
                            )
                            nc.tensor.matmul(
                                sl,
                                ex[:, h, 256:384],
                                vp[:, 1, h * 65 : h * 65 + 65],
                                start=False,
                                stop=True,
                            )
                    rc = rcp.tile([128, H], FP, tag="rc")
                    pav3 = pav[:].rearrange("p (h c) -> p h c", c=65)
                    nc.vector.reciprocal(rc[:], pav3[:, :, 64])
                    if NORM == "bcast":
                        rb = rc[:].unsqueeze(2).broadcast_to((128, H, 64))
                        nc.vector.tensor_mul(
                            oc[:, tc_, :].rearrange("p (h d) -> p h d", d=64),
                            pav3[:, :, 0:64],
                            rb,
                        )
                    else:
                        for h in range(H):
                            nc.vector.tensor_scalar_mul(
                                oc[:, tc_, h * 64 : h * 64 + 64],
                                pav3[:, h, 0:64],
                                rc[:, h : h + 1],
                            )


                # ---- oT via PE transpose + output projection ----
                for tc_ in range(2):
                    oT = oTp.tile([128, 3, 128], BF, tag=f"oT{tc_}", name=f"oT{tc_}")
                    for hc in range(3):
                        pt = pst.tile([128, 128], BF, tag="pst")
                        nc.tensor.transpose(
                            pt[:], oc[:, tc_, hc * 128 : hc * 128 + 128], id_t[:]
                        )
                        nc.vector.tensor_copy(oT[:, hc, :], pt[:])
                    py = ps.tile([128, 512], FP, tag="ps", name="py")
                    for hc in range(3):
                        nc.tensor.matmul(
                            py[:, 0:E],
                            oT[:, hc, :],
                            wo_t[:, hc, :],
                            start=(hc == 0),
                            stop=(hc == 2),
                        )
                    fin = fip.tile([128, E], FP, tag="fin")
                    nc.vector.tensor_add(fin[:], py[:, 0:E], bo_t[:])
                    nc.sync.dma_start(
                        y_d[b * T + tc_ * 128 : b * T + tc_ * 128 + 128, :], fin[:]
                    )

            def _batch_loop():
                for b in range(n_batches):
                    _one_batch(b)

            if reps == 1:
                _batch_loop()
            else:
                with tc.For_i(0, reps, 1):
                    _batch_loop()

    nc.finalize()
    return nc


def _host_inputs(x, Wq, Wk, Wv, Wo, bo):
    import ml_dtypes

    bf = ml_dtypes.bfloat16
    # x transposed per batch on host: [B, E, T]
    xT = np.ascontiguousarray(
        np.asarray(x, dtype=np.float32).transpose(0, 2, 1)
    ).astype(bf)
    wq = np.ascontiguousarray(
        np.asarray(Wq, dtype=np.float32).transpose(1, 0, 2).reshape(E, HD)
    ).astype(bf)
    wk = np.ascontiguousarray(
        np.asarray(Wk, dtype=np.float32).transpose(1, 0, 2).reshape(E, HD)
    ).astype(bf)
    wv = np.ascontiguousarray(
        np.asarray(Wv, dtype=np.float32).transpose(1, 0, 2).reshape(E, HD)
    ).astype(bf)
    wo = np.ascontiguousarray(np.asarray(Wo, dtype=np.float32)).astype(bf)
    bo_rep = np.ascontiguousarray(
        np.tile(np.asarray(bo, dtype=np.float32).reshape(1, E), (128, 1))
    )
    mask = np.triu(np.ones((128, 128), dtype=np.float32)).astype(bf)
    ident = np.eye(128, dtype=np.float32).astype(bf)
    return xT, wq, wk, wv, wo, bo_rep, mask, ident


def kernel(x, Wq, Wk, Wv, Wo, bo, _trace=False, _n_batches=BL, _reps=1):
    from concourse import bass_utils

    xT, wq, wk, wv, wo, bo_rep, mask, ident = _host_inputs(x, Wq, Wk, Wv, Wo, bo)

    nc = _build_program(_n_batches, _reps)
    in_maps = []
    for c in range(N_CORES):
        xs = xT[c * BL : c * BL + _n_batches].reshape(_n_batches * E, T)
        in_maps.append(
            {
                "x": np.ascontiguousarray(xs),
                "wq": wq,
                "wk": wk,
                "wv": wv,
                "wo": wo,
                "bo": bo_rep,
                "mask": mask,
                "ident": ident,
            }
        )
    res = bass_utils.run_bass_kernel_spmd(
        nc, in_maps, core_ids=list(range(N_CORES)), trace=_trace
    )
    y = np.concatenate(
        [r["y"].reshape(_n_batches, T, E) for r in res.results], axis=0
    ).astype(np.float32)
    if _trace:
        return y, res
    return y


# revision 29
# speedup vs baseline: 1.5787x; 1.5787x over previous
"""Multi-head causal self-attention on 8 Trainium2 NeuronCores.

Problem: B=256, T=256, E=384, H=6, D=64 (fp32 in/out).
Strategy: pure data parallelism over the batch dim - each of the 8 cores
processes 32 batches end-to-end. No collectives.

v3 design:
- all matmul operands bf16 (1 cyc/row on PE at any free-dim size, fp32
  PSUM accumulation); inputs pre-cast host-side.
- x^T computed on the HOST (free) and DMA'd in directly - no on-device
  x transpose at all.
- o^T via PE transpose (bf16, 128 cyc) + DVE copy; DMA xbar transposes
  proved to serialize the whole DMA path (xbar-mode thrash), so none.
- scores: per-head PSUM bank [s0 256 | s1 128] -> single fused exp per
  head on ACT (scale=1/8, bf16 out). The two MMs of one head share a PE
  row-group so they serialize; concurrent different-row-group MMs
  (adjacent heads, partition offsets 0/64) land in different banks.
  (Concurrent MMs into one PSUM bank crash the HW - found the hard way.)
- causal mask on diag blocks via 2 broadcast tensor_tensor muls (DVE).
- softmax denominator via packed [v_h | 1] rhs (65th column) in the av
  matmul; normalization fused into the PSUM->SBUF copy as a broadcast
  tensor_tensor mul with the per-head reciprocal.
- q/k/v/proj PSUM banks packed 2 chunks per bank (K=128 MMs serialize).

Softmax max-subtraction skipped deliberately: |scores| < ~2 given the
input distribution, exp() is well-conditioned in fp32->bf16.
"""

import os
import sys

import numpy as np

sys.path.insert(0, "/opt/trn_rl_repo")

B, T, E, H, D = 256, 256, 384, 6, 64
HD = H * D  # 384
N_CORES = 8
BL = B // N_CORES  # 32 batches per core

NORM = os.environ.get("K_NORM", "bcast")  # bcast | ts


def _build_program(n_batches=BL, reps=1):
    import concourse.mybir as mybir
    import concourse.tile as tile
    from concourse import bacc

    FP = mybir.dt.float32
    BF = mybir.dt.bfloat16
    AF = mybir.ActivationFunctionType

    nc = bacc.Bacc(
        "TRN2",
        target_bir_lowering=False,
        debug=False,
        enable_asserts=False,
        num_devices=N_CORES,
        enable_partition_id=False,
    )

    # x arrives pre-transposed per batch: [nb*E, T]
    x_d = nc.dram_tensor("x", (n_batches * E, T), BF, kind="ExternalInput").ap()
    wq_d = nc.dram_tensor("wq", (E, HD), BF, kind="ExternalInput").ap()
    wk_d = nc.dram_tensor("wk", (E, HD), BF, kind="ExternalInput").ap()
    wv_d = nc.dram_tensor("wv", (E, HD), BF, kind="ExternalInput").ap()
    wo_d = nc.dram_tensor("wo", (HD, E), BF, kind="ExternalInput").ap()
    bo_d = nc.dram_tensor("bo", (128, E), FP, kind="ExternalInput").ap()
    mk_d = nc.dram_tensor("mask", (128, 128), BF, kind="ExternalInput").ap()
    id_d = nc.dram_tensor("ident", (128, 128), BF, kind="ExternalInput").ap()
    y_d = nc.dram_tensor("y", (n_batches * T, E), FP, kind="ExternalOutput").ap()

    with tile.TileContext(nc) as tc:
        from contextlib import ExitStack

        with ExitStack() as ctx:
            const = ctx.enter_context(tc.tile_pool(name="const", bufs=1))
            wq_t = const.tile([128, 3, HD], BF, tag="wq")
            wk_t = const.tile([128, 3, HD], BF, tag="wk")
            wv_t = const.tile([128, 3, HD], BF, tag="wv")
            wo_t = const.tile([128, 3, E], BF, tag="wo")
            bo_t = const.tile([128, E], FP, tag="bo")
            mk_t = const.tile([128, 128], BF, tag="mask")
            id_t = const.tile([128, 128], BF, tag="ident")
            # SBUF pools
            xTp = ctx.enter_context(tc.tile_pool(name="xT", bufs=3))

            # wq + batch-0 xT first so the first q matmuls start ASAP;
            # remaining constants follow (needed later in the pipeline).
            nc.sync.dma_start(wq_t[:], wq_d.rearrange("(c p) n -> p c n", p=128))
            xT0 = xTp.tile([128, 3, T], BF, tag="xT", name="xT0")
            nc.sync.dma_start(
                xT0[:], x_d[0:E, :].rearrange("(c p) n -> p c n", p=128)
            )
            for t_, d_ in ((wk_t, wk_d), (wv_t, wv_d), (wo_t, wo_d)):
                nc.sync.dma_start(t_[:], d_.rearrange("(c p) n -> p c n", p=128))
            nc.sync.dma_start(bo_t[:], bo_d)
            nc.sync.dma_start(mk_t[:], mk_d)
            nc.sync.dma_start(id_t[:], id_d)
            mk_b = mk_t[:].unsqueeze(1).broadcast_to((128, H, 128))
            qkp = ctx.enter_context(tc.tile_pool(name="qk", bufs=2))
            vpp = ctx.enter_context(tc.tile_pool(name="vp", bufs=2))
            exp = ctx.enter_context(tc.tile_pool(name="ex", bufs=2))
            rcp = ctx.enter_context(tc.tile_pool(name="rc", bufs=2))
            ocp = ctx.enter_context(tc.tile_pool(name="oc", bufs=2))
            oTp = ctx.enter_context(tc.tile_pool(name="oT", bufs=2))
            fip = ctx.enter_context(tc.tile_pool(name="fi", bufs=3))

            # PSUM pools: 4 + 2 + 2 = 8 banks
            ps = ctx.enter_context(tc.tile_pool(name="ps", bufs=4, space="PSUM"))
            ps2 = ctx.enter_context(tc.tile_pool(name="ps2", bufs=2, space="PSUM"))
            pst = ctx.enter_context(tc.tile_pool(name="pst", bufs=2, space="PSUM"))

            def _one_batch(b):
                # ---- xT [e, t] bf16: plain DMA load (pre-transposed on host) ----
                if b == 0:
                    xTt = xT0
                else:
                    xTt = xTp.tile([128, 3, T], BF, tag="xT")
                    nc.sync.dma_start(
                        xTt[:],
                        x_d[b * E : (b + 1) * E, :].rearrange(
                            "(c p) n -> p c n", p=128
                        ),
                    )

                # ---- q/k projections (PSUM banks packed 2 chunks each) ----
                qT = qkp.tile([128, 3, T], BF, tag="qT")
                kT = qkp.tile([128, 3, T], BF, tag="kT")
                pA = ps.tile([128, 512], FP, tag="ps")  # q hc0 | q hc1
                pB = ps.tile([128, 512], FP, tag="ps")  # q hc2 | k hc0
                pC = ps.tile([128, 512], FP, tag="ps")  # k hc1 | k hc2
                mm_plan = [
                    (pA, 0, wq_t, 0),
                    (pA, 1, wq_t, 1),
                    (pB, 0, wq_t, 2),
                    (pB, 1, wk_t, 0),
                    (pC, 0, wk_t, 1),
                    (pC, 1, wk_t, 2),
                ]
                for pt_, half, w_t, hc in mm_plan:
                    for ec in range(3):
                        nc.tensor.matmul(
                            pt_[:, half * T : half * T + T],
                            w_t[:, ec, hc * 128 : hc * 128 + 128],
                            xTt[:, ec, :],
                            start=(ec == 0),
                            stop=(ec == 2),
                        )
                # copy-cast PSUM->SBUF bf16 on ACT
                nc.scalar.copy(qT[:, 0:2, :], pA[:])
                nc.scalar.copy(qT[:, 2, :], pB[:, 0:T])
                nc.scalar.copy(kT[:, 0, :], pB[:, T : 2 * T])
                nc.scalar.copy(kT[:, 1:3, :], pC[:])

                # ---- v projection -> packed [v_h | 1] per head ----
                vp = vpp.tile([128, 2, H * 65], BF, tag="vp")
                for sc in range(2):
                    pv = ps2.tile([128, 390], FP, tag="ps2")
                    for ec in range(3):
                        nc.tensor.matmul(
                            pv[:, 0:HD],
                            xTt[:, ec, sc * 128 : sc * 128 + 128],
                            wv_t[:, ec, :],
                            start=(ec == 0),
                            stop=(ec == 2),
                        )
                    dst = vp[:, sc, :].rearrange("p (h c) -> p h c", c=65)
                    nc.vector.tensor_copy(
                        dst[:, :, 0:64], pv[:, 0:HD].rearrange("p (h d) -> p h d", d=64)
                    )
                    nc.vector.memset(dst[:, :, 64:65], 1.0)


                # ---- scores + exp: per-head PSUM bank [s0 256 | s1 128] ----
                # Both MMs of a head share a row-group (same po) so they
                # serialize on the PE; concurrent different-row-group MMs
                # (adjacent heads) land in different banks. Concurrent MMs
                # into one bank crash the HW.
                ex = exp.tile([128, H, 384], BF, tag="ex")
                for h in range(H):
                    hc, po = divmod(h, 2)
                    po *= 64
                    sb = ps.tile([128, 512], FP, tag="ps", name=f"sb{h}")
                    nc.tensor.matmul(
                        sb[:, 0:T],
                        kT[po : po + 64, hc, 0:128],
                        qT[po : po + 64, hc, :],
                        start=True,
                        stop=True,
                    )
                    nc.tensor.matmul(
                        sb[:, T : T + 128],
                        kT[po : po + 64, hc, 128:256],
                        qT[po : po + 64, hc, 128:256],
                        start=True,
                        stop=True,
                    )
                    nc.scalar.activation(
                        ex[:, h, :], sb[:, 0 : T + 128], AF.Exp, scale=0.125
                    )
                # causal mask on diagonal blocks (s0 diag at cols 0:128,
                # s1 diag at cols 256:384)
                nc.vector.tensor_mul(ex[:, :, 0:128], ex[:, :, 0:128], mk_b)
                nc.vector.tensor_mul(ex[:, :, 256:384], ex[:, :, 256:384], mk_b)


                # ---- av + normalize -> oc[t, hd] ----
                oc = ocp.tile([128, 2, HD], BF, tag="oc")
                for tc_ in range(2):
                    pav = ps2.tile([128, 390], FP, tag="ps2")
                    for h in range(H):
                        sl = pav[:, h * 65 : h * 65 + 65]
                        if tc_ == 0:
                            nc.tensor.matmul(
                                sl,
                                ex[:, h, 0:128],
                                vp[:, 0, h * 65 : h * 65 + 65],
                                start=True,
                                stop=True,
                            )
                        else:
                            nc.tensor.matmul(
                                sl,
                                ex[:, h, 128:256],
                                vp[:, 0, h * 65 : h * 65 + 65],
                                start=True,
                                stop=False,
                            )
                            nc.tensor.matmul(
                                sl,
                                ex[:, h, 256:384],
                                vp[:, 1, h * 65 : h * 65 + 65],
                                start=False,
                                stop=True,
                            )
                    rc = rcp.tile([128, H], FP, tag="rc")
                    pav3 = pav[:].rearrange("p (h c) -> p h c", c=65)
                    nc.vector.reciprocal(rc[:], pav3[:, :, 64])
                    if NORM == "bcast":
                        rb = rc[:].unsqueeze(2).broadcast_to((128, H, 64))
                        nc.vector.tensor_mul(
                            oc[:, tc_, :].rearrange("p (h d) -> p h d", d=64),
                            pav3[:, :, 0:64],
                            rb,
                        )
                    else:
                        for h in range(H):
                            nc.vector.tensor_scalar_mul(
                                oc[:, tc_, h * 64 : h * 64 + 64],
                                pav3[:, h, 0:64],
                                rc[:, h : h + 1],
                            )


                # ---- oT via PE transpose + output projection ----
                for tc_ in range(2):
                    oT = oTp.tile([128, 3, 128], BF, tag=f"oT{tc_}", name=f"oT{tc_}")
                    for hc in range(3):
                        pt = pst.tile([128, 128], BF, tag="pst")
                        nc.tensor.transpose(
                            pt[:], oc[:, tc_, hc * 128 : hc * 128 + 128], id_t[:]
                        )
                        nc.scalar.copy(oT[:, hc, :], pt[:])
                    py = ps2.tile([128, 390], FP, tag="ps2")
                    for hc in range(3):
                        nc.tensor.matmul(
                            py[:, 0:E],
                            oT[:, hc, :],
                            wo_t[:, hc, :],
                            start=(hc == 0),
                            stop=(hc == 2),
                        )
                    fin = fip.tile([128, E], FP, tag="fin")
                    nc.vector.tensor_add(fin[:], py[:, 0:E], bo_t[:])
                    nc.sync.dma_start(
                        y_d[b * T + tc_ * 128 : b * T + tc_ * 128 + 128, :], fin[:]
                    )

            def _batch_loop():
                for b in range(n_batches):
                    _one_batch(b)

            if reps == 1:
                _batch_loop()
            else:
                with tc.For_i(0, reps, 1):
                    _batch_loop()

    nc.finalize()
    return nc


def _host_inputs(x, Wq, Wk, Wv, Wo, bo):
    import ml_dtypes

    bf = ml_dtypes.bfloat16
    # x transposed per batch on host: [B, E, T]
    xT = np.ascontiguousarray(
        np.asarray(x, dtype=np.float32).transpose(0, 2, 1)
    ).astype(bf)
    wq = np.ascontiguousarray(
        np.asarray(Wq, dtype=np.float32).transpose(1, 0, 2).reshape(E, HD)
    ).astype(bf)
    wk = np.ascontiguousarray(
        np.asarray(Wk, dtype=np.float32).transpose(1, 0, 2).reshape(E, HD)
    ).astype(bf)
    wv = np.ascontiguousarray(
        np.asarray(Wv, dtype=np.float32).transpose(1, 0, 2).reshape(E, HD)
    ).astype(bf)
    wo = np.ascontiguousarray(np.asarray(Wo, dtype=np.float32)).astype(bf)
    bo_rep = np.ascontiguousarray(
        np.tile(np.asarray(bo, dtype=np.float32).reshape(1, E), (128, 1))
    )
    mask = np.triu(np.ones((128, 128), dtype=np.float32)).astype(bf)
    ident = np.eye(128, dtype=np.float32).astype(bf)
    return xT, wq, wk, wv, wo, bo_rep, mask, ident


def kernel(x, Wq, Wk, Wv, Wo, bo, _trace=False, _n_batches=BL, _reps=1):
    from concourse import bass_utils

    xT, wq, wk, wv, wo, bo_rep, mask, ident = _host_inputs(x, Wq, Wk, Wv, Wo, bo)

    nc = _build_program(_n_batches, _reps)
    in_maps = []
    for c in range(N_CORES):
        xs = xT[c * BL : c * BL + _n_batches].reshape(_n_batches * E, T)
        in_maps.append(
            {
                "x": np.ascontiguousarray(xs),
                "wq": wq,
                "wk": wk,
                "wv": wv,
                "wo": wo,
                "bo": bo_rep,
                "mask": mask,
                "ident": ident,
            }
        )
    res = bass_utils.run_bass_kernel_spmd(
        nc, in_maps, core_ids=list(range(N_CORES)), trace=_trace
    )
    y = np.concatenate(
        [r["y"].reshape(_n_batches, T, E) for r in res.results], axis=0
    ).astype(np.float32)
    if _trace:
        return y, res
    return y
